# revision 20
# baseline (speedup 1.0000x reference)
"""GAT layer kernel for Trainium2, data-parallel over batch across 8 NeuronCores.

Per core (one batch b), with s1 = Wh@a1, s2 = Wh@a2, alpha the LeakyReLU slope:

  E_ij = exp(leaky_relu(s1_i + s2_j)) = e^{s1_i} * max(e^{s2_j}, e^{(a-1)s1_i} e^{a s2_j})

The e^{s1_i} factor cancels in softmax(E)_ij = M_ij / S_i, where

  M'_ji = (g_i max k_j) * Q_j        g_i = e^{(a-1)s1_i}  (broadcast tensor)
                                     k_j = e^{(1-a)s2_j}, Q_j = e^{a s2_j} (scalars)
  S_i   = sum_j M'_ji

M' is laid out with partition = column node j, free = row node i, so each j-tile
chunk is ONE DVE tensor_scalar (max,mult with two per-partition scalars) in bf16
at 4x rate.  PE consumes each M' chunk with stationary [Wh_t | 1], producing the
unnormalized output y_fi = sum_j Wh_jf M'_ji and S_i in one 65-row PSUM stream.
Inputs are pre-converted to bf16 on the host; hT comes from DRAM in a single
XBAR DMA transpose.  [Wh_t | s2_t] come from one matmul per tile against the
moving operand [W | w2], w2 = W@a2.  Work runs in 4 column-chunk blocks; each y
PSUM bank completes at the end of its block, so the normalization epilogue
(out_f = (1/N) sum_i y_fi / S_i, via PE transpose + small matmuls) pipelines
per bank behind the next block.
"""
import sys
sys.path.insert(0, "/opt/trn_rl_repo")
from contextlib import ExitStack

import ml_dtypes
import numpy as np

import concourse.bass as bass
import concourse.tile as tile
from concourse import bacc, mybir
from concourse.bass_utils import run_bass_kernel_spmd
from concourse.masks import make_identity

N, K, F, P, T = 2048, 128, 64, 128, 16  # nodes, f_in, f_out, partitions, j tiles
NC_ = 4            # 512-wide column chunks per N
ALPHA = 0.2
NCORES = 8
FP = mybir.dt.float32
BF = mybir.dt.bfloat16
AF = mybir.ActivationFunctionType
OP = mybir.AluOpType
ts = bass.ts
FO = F + 1
WPC = FO + K + 2  # packed W-side input columns
HEXT = ((N + WPC + 15) // 16) * 16  # h rows + wp rows, padded for the XBAR


def emit_batch(tc, outd, hb):
    """Emit the full per-batch computation (one repetition)."""
    nc = tc.nc
    with ExitStack() as ctx:
        big = ctx.enter_context(tc.tile_pool(name="big", bufs=1))
        mpool = ctx.enter_context(tc.tile_pool(name="m", bufs=1))
        small = ctx.enter_context(tc.tile_pool(name="small", bufs=1))
        ypsum_ctx = tc.tile_pool(name="yps", bufs=1, space=bass.MemorySpace.PSUM)
        ypsum = ypsum_ctx.__enter__()
        pro_psum_ctx = tc.tile_pool(name="ps", bufs=1, space=bass.MemorySpace.PSUM)
        psum = pro_psum_ctx.__enter__()

        # ONE XBAR DMA transpose delivers hT[k, n] AND the packed W-side data
        # [W | w2slot | W^T | a12] (appended as extra h rows on the host)
        hTx = big.tile([P, HEXT], BF)
        nc.sync.dma_start_transpose(hTx[:], hb[:, :])
        hT = hTx[:, 0:N]
        wpack = hTx[:, N : N + WPC]
        Wv = wpack[:, 0:FO]               # [W | w2] moving operand (w2 filled below)
        WT_bf = hTx[0:F, N + FO : N + FO + K]   # W^T
        a12_bf = hTx[0:F, N + FO + K : N + WPC]  # [a1 | a2]

        # junk tile: PE clock-warm operand, ACT exp-table warm, shape donor
        junk = small.tile([P, 512], BF, tag="junk")
        nc.vector.memset(junk[:], 1.0)
        warm1 = small.tile([P, 1], FP, tag="warm1")
        nc.scalar.activation(warm1[:], junk[:, 0:1], AF.Exp)
        warm_ps = psum.tile([P, 512], FP, tag="s1b", bufs=1, name="ps_warm")
        for _ in range(6):
            nc.tensor.matmul(warm_ps[:], junk[0:P, 0:P], junk[:], start=True, stop=True)

        ident = small.tile([P, P], FP, tag="ident")
        make_identity(nc, ident[:])

        # W prep: [w1|w2] = (W^T)^T @ [a1|a2], w1_rep, w2 into the Wv slot
        wprep_ps = psum.tile([P, 4 * FO], FP, tag="wh", bufs=2, name="ps_wprep")
        nc.tensor.matmul(wprep_ps[:, 0:2], WT_bf, a12_bf, start=True, stop=True)
        w12_sb = small.tile([K, 2], FP, tag="w12")
        nc.scalar.copy(w12_sb[:], wprep_ps[:, 0:2])
        w1_rep = small.tile([K, P], BF, tag="w1rep")
        nc.scalar.activation(w1_rep[:], junk[0:K, 0:P], AF.Identity, bias=w12_sb[:, 0:1], scale=0.0)
        nc.vector.tensor_copy(hTx[:, N + F : N + FO], w12_sb[:, 1:2])

        g_b = big.tile([P, N], BF)      # g_b[p, i] = e^{(a-1) s1_i}  (any p)
        kcol = small.tile([P, T], FP, tag="kcol")   # k_j column layout
        qcol = small.tile([P, T], FP, tag="qcol")   # Q_j column layout
        kcol3 = kcol[:].rearrange("p (t one) -> p t one", one=1)
        qcol3 = qcol[:].rearrange("p (t one) -> p t one", one=1)
        WhO = big.tile([P, T * FO], BF)             # [Wh_t | ones] blocks
        WhO3 = WhO[:].rearrange("p (t c) -> p t c", c=FO)
        nc.vector.memset(WhO3[:, :, F:FO], 1.0)

        y_ps = [
            ypsum.tile([FO, 512], FP, tag=f"y{c}", name=f"y_ps{c}") for c in range(NC_)
        ]
        Ms = [mpool.tile([P, N], BF, tag=f"M{t}", name=f"M{t}") for t in range(T)]

        # epilogue tiles (filled per bank as each completes)
        y_sb = small.tile([FO, N], FP, tag="ysb")
        yT_sb = small.tile([P, T * FO], BF, tag="yTsb")
        yT3 = yT_sb[:].rearrange("p (t c) -> p t c", c=FO)
        invS = small.tile([P, T], FP, tag="invS")
        invS3 = invS[:].rearrange("p (t one) -> p t one", one=1)
        invS_bf = small.tile([P, T], BF, tag="invSbf")

        g_ps = y_ps[0][0:F, 511:512]

        def epi_bank(g):
            """Normalization pipeline for completed y bank g."""
            if g % 2 == 0:
                nc.scalar.copy(y_sb[:, ts(g, 512)], y_ps[g][:])
            else:
                nc.vector.tensor_copy(y_sb[:, ts(g, 512)], y_ps[g][:])
            yt_ps = psum.tile([P, 4 * FO], FP, tag="yt", bufs=1, name="ps_yt")
            for q in range(4):
                nc.tensor.transpose(
                    yt_ps[:, q * FO : (q + 1) * FO],
                    y_sb[:, ts(4 * g + q, P)], ident[0:FO, 0:FO],
                )
            nc.scalar.copy(yT_sb[:, g * 4 * FO : (g + 1) * 4 * FO], yt_ps[:])
            nc.vector.reciprocal(invS3[:, 4 * g : 4 * g + 4, :], yT3[:, 4 * g : 4 * g + 4, F:FO])
            nc.vector.tensor_copy(invS_bf[:, 4 * g : 4 * g + 4], invS[:, 4 * g : 4 * g + 4])
            for t in range(4 * g, 4 * g + 4):
                nc.tensor.matmul(
                    g_ps, yT_sb[:, t * FO : t * FO + F], invS_bf[:, t : t + 1],
                    start=(t == 0), stop=(t == T - 1), skip_group_check=True,
                )

        # s1 broadcast -> g_b, all four chunks up front (gated only on hT+w1_rep)
        for g in range(4):
            s1b_ps = psum.tile([P, 512], FP, tag="s1b", bufs=1, name="ps_s1b")
            nc.tensor.matmul(s1b_ps[:], w1_rep[:], hT[:, ts(g, 512)], start=True, stop=True)
            nc.scalar.activation(g_b[:, ts(g, 512)], s1b_ps[:], AF.Exp, scale=ALPHA - 1.0)

        # [Wh_t | s2_t] slabs: one matmul per tile against [W | w2]
        for g in range(4):
            wh_ps = psum.tile([P, 4 * FO], FP, tag="wh", bufs=2, name="ps_wh")
            wh3 = wh_ps[:].rearrange("p (q c) -> p q c", c=FO)
            for q in range(4):
                t = 4 * g + q
                nc.tensor.matmul(
                    wh3[:, q, :], hT[:, ts(t, P)], Wv, start=True, stop=True
                )
            nc.scalar.copy(WhO3[:, 4 * g : 4 * g + 4, 0:F], wh3[:, :, 0:F])
            nc.scalar.activation(kcol3[:, 4 * g : 4 * g + 4, :], wh3[:, :, F:FO], AF.Exp, scale=1.0 - ALPHA)
            nc.scalar.activation(qcol3[:, 4 * g : 4 * g + 4, :], wh3[:, :, F:FO], AF.Exp, scale=ALPHA)

        # main N^2 stream: 4 chunk blocks of 16 DVE tensor_scalar + 16 PE matmuls
        for g in range(4):
            for t in range(T):
                nc.vector.tensor_scalar(
                    Ms[t][:, ts(g, 512)], g_b[:, ts(g, 512)],
                    kcol[:, t : t + 1], qcol[:, t : t + 1], OP.max, OP.mult,
                )
            for t in range(T):
                nc.tensor.matmul(
                    y_ps[g][0:FO, :], WhO3[:, t, :], Ms[t][:, ts(g, 512)],
                    start=(t == 0), stop=(t == T - 1),
                )
            if g >= 1:
                epi_bank(g - 1)

        epi_bank(3)
        pro_psum_ctx.__exit__(None, None, None)
        out_sb = small.tile([F, 1], FP, tag="out")
        nc.scalar.mul(out_sb[:], g_ps, 1.0 / N)
        nc.sync.dma_start(outd[:], out_sb[:])
        ypsum_ctx.__exit__(None, None, None)


def build(reps: int = 1):
    nc = bacc.Bacc(
        "TRN2", target_bir_lowering=False, debug=False,
        enable_asserts=False, num_devices=NCORES,
    )
    hb = nc.dram_tensor("hb", [HEXT, K], BF, kind="ExternalInput").ap()
    outd = nc.dram_tensor("out", [F, 1], FP, kind="ExternalOutput").ap()

    with tile.TileContext(nc) as tc:
        for _ in range(reps):
            emit_batch(tc, outd, hb)
    nc.compile()
    return nc


_nc_cache = {}


def _get_nc(reps: int = 1):
    if reps not in _nc_cache:
        _nc_cache[reps] = build(reps)
    return _nc_cache[reps]


def _pack_inputs(hb: np.ndarray, W: np.ndarray, a: np.ndarray) -> np.ndarray:
    """Host-side staging: h plus the packed W-side data as extra rows, bf16."""
    wp = np.zeros((K, WPC), dtype=np.float32)
    wp[:, 0:F] = W
    wp[0:F, FO : FO + K] = W.T
    wp[0:F, FO + K] = a[:F]
    wp[0:F, FO + K + 1] = a[F:]
    ext = np.zeros((HEXT, K), dtype=np.float32)
    ext[0:N] = hb
    ext[N : N + WPC] = wp.T
    return ext.astype(ml_dtypes.bfloat16)


def kernel(h: np.ndarray, W: np.ndarray, a: np.ndarray) -> np.ndarray:
    assert h.shape == (NCORES, N, K) and W.shape == (K, F) and a.shape == (2 * F,)
    nc = _get_nc(1)
    in_maps = [{"hb": _pack_inputs(h[b], W, a)} for b in range(NCORES)]
    res = run_bass_kernel_spmd(nc, in_maps, core_ids=list(range(NCORES)))
    out = np.stack([res.results[b]["out"].reshape(F) for b in range(NCORES)])
    return out.astype(np.float32)


# revision 27
# speedup vs baseline: 1.0284x; 1.0284x over previous
"""GAT layer kernel for Trainium2, data-parallel over batch across 8 NeuronCores.

Per core (one batch b), with s1 = Wh@a1, s2 = Wh@a2, alpha the LeakyReLU slope:

  E_ij = exp(leaky_relu(s1_i + s2_j)) = e^{s1_i} * max(e^{s2_j}, e^{(a-1)s1_i} e^{a s2_j})

The e^{s1_i} factor cancels in softmax(E)_ij = M_ij / S_i, where

  M'_ji = (g_i max k_j) * Q_j        g_i = e^{(a-1)s1_i}  (broadcast tensor)
                                     k_j = e^{(1-a)s2_j}, Q_j = e^{a s2_j} (scalars)
  S_i   = sum_j M'_ji

M' is laid out with partition = column node j, free = row node i, so each j-tile
chunk is ONE DVE tensor_scalar (max,mult with two per-partition scalars) in bf16
at 4x rate.  PE consumes each M' chunk with stationary [Wh_t | 1], producing the
unnormalized output y_fi = sum_j Wh_jf M'_ji and S_i in one 65-row PSUM stream.
Inputs are pre-converted to bf16 on the host; hT comes from DRAM in a single
XBAR DMA transpose.  [Wh_t | s2_t] come from one matmul per tile against the
moving operand [W | w2], w2 = W@a2.  Work runs in 4 column-chunk blocks; each y
PSUM bank completes at the end of its block, so the normalization epilogue
(out_f = (1/N) sum_i y_fi / S_i, via PE transpose + small matmuls) pipelines
per bank behind the next block.
"""
import sys
sys.path.insert(0, "/opt/trn_rl_repo")
from contextlib import ExitStack

import ml_dtypes
import numpy as np

import concourse.bass as bass
import concourse.tile as tile
from concourse import bacc, mybir
from concourse.bass_utils import run_bass_kernel_spmd
from concourse.masks import make_identity

N, K, F, P, T = 2048, 128, 64, 128, 16  # nodes, f_in, f_out, partitions, j tiles
NC_ = 4            # 512-wide column chunks per N
ALPHA = 0.2
NCORES = 8
FP = mybir.dt.float32
BF = mybir.dt.bfloat16
AF = mybir.ActivationFunctionType
OP = mybir.AluOpType
ts = bass.ts
FO = F + 1
WPC = FO + K + 2  # packed W-side input columns
WPAD = ((WPC + 15) // 16) * 16      # wp rows padded for the XBAR
HEXT = WPAD + N                     # wp rows first, then h rows
HCUT = WPAD + N // 2                # first-DMA row count (wp + h chunks 0-1)


def emit_batch(tc, outd, hb):
    """Emit the full per-batch computation (one repetition)."""
    nc = tc.nc
    with ExitStack() as ctx:
        big = ctx.enter_context(tc.tile_pool(name="big", bufs=1))
        mpool = ctx.enter_context(tc.tile_pool(name="m", bufs=1))
        small = ctx.enter_context(tc.tile_pool(name="small", bufs=1))
        ypsum_ctx = tc.tile_pool(name="yps", bufs=1, space=bass.MemorySpace.PSUM)
        ypsum = ypsum_ctx.__enter__()
        pro_psum_ctx = tc.tile_pool(name="ps", bufs=1, space=bass.MemorySpace.PSUM)
        psum = pro_psum_ctx.__enter__()

        # TWO XBAR DMA transposes deliver the packed W-side data
        # [W | w2slot | W^T | a12] (prepended as extra h rows on the host)
        # plus h chunks 0-1, then h chunks 2-3 — so the W-prep chain and the
        # first blocks start while the second half is still in flight
        hTx = big.tile([P, HEXT], BF)
        nc.sync.dma_start_transpose(hTx[:, 0:HCUT], hb[0:HCUT, :])
        nc.scalar.dma_start_transpose(hTx[:, HCUT:HEXT], hb[HCUT:HEXT, :])
        hT = hTx[:, WPAD : WPAD + N]
        Wv = hTx[:, 0:FO]                 # [W | w2] moving operand (w2 filled below)
        WT_bf = hTx[0:F, FO : FO + K]     # W^T
        a12_bf = hTx[0:F, FO + K : WPC]   # [a1 | a2]

        # junk tile: PE clock-warm operand, ACT exp-table warm, shape donor
        junk = small.tile([P, 512], BF, tag="junk")
        nc.vector.memset(junk[:], 1.0)
        warm1 = small.tile([P, 1], FP, tag="warm1")
        nc.scalar.activation(warm1[:], junk[:, 0:1], AF.Exp)
        warm_ps = psum.tile([P, 512], FP, tag="s1b", bufs=1, name="ps_warm")
        for _ in range(4):
            nc.tensor.matmul(warm_ps[:], junk[0:P, 0:P], junk[:], start=True, stop=True)

        ident = small.tile([P, P], FP, tag="ident")
        make_identity(nc, ident[:])

        # W prep: [w1|w2] = (W^T)^T @ [a1|a2], w1_rep, w2 into the Wv slot
        wprep_ps = psum.tile([P, 4 * FO], FP, tag="wh", bufs=2, name="ps_wprep")
        nc.tensor.matmul(wprep_ps[:, 0:2], WT_bf, a12_bf, start=True, stop=True)
        w12_sb = small.tile([K, 2], FP, tag="w12")
        nc.scalar.copy(w12_sb[:], wprep_ps[:, 0:2])
        w1_rep = small.tile([K, P], BF, tag="w1rep")
        nc.scalar.activation(w1_rep[:], junk[0:K, 0:P], AF.Identity, bias=w12_sb[:, 0:1], scale=0.0)
        nc.vector.tensor_copy(hTx[:, F:FO], w12_sb[:, 1:2])

        g_b = big.tile([P, N], BF)      # g_b[p, i] = e^{(a-1) s1_i}  (any p)
        kcol = small.tile([P, T], FP, tag="kcol")   # k_j column layout
        qcol = small.tile([P, T], FP, tag="qcol")   # Q_j column layout
        kcol3 = kcol[:].rearrange("p (t one) -> p t one", one=1)
        qcol3 = qcol[:].rearrange("p (t one) -> p t one", one=1)
        WhO = big.tile([P, T * FO], BF)             # [Wh_t | ones] blocks
        WhO3 = WhO[:].rearrange("p (t c) -> p t c", c=FO)
        nc.vector.memset(WhO3[:, :, F:FO], 1.0)

        y_ps = [
            ypsum.tile([FO, 512], FP, tag=f"y{c}", name=f"y_ps{c}") for c in range(NC_)
        ]
        Ms = [mpool.tile([P, N], BF, tag=f"M{t}", name=f"M{t}") for t in range(T)]

        # epilogue tiles (filled per bank as each completes)
        y_sb = small.tile([FO, N], FP, tag="ysb")
        yT_sb = small.tile([P, T * FO], BF, tag="yTsb")
        yT3 = yT_sb[:].rearrange("p (t c) -> p t c", c=FO)
        invS = small.tile([P, T], FP, tag="invS")
        invS3 = invS[:].rearrange("p (t one) -> p t one", one=1)
        invS_bf = small.tile([P, T], BF, tag="invSbf")

        g_ps = y_ps[0][0:F, 511:512]

        def epi_bank(g):
            """Normalization pipeline for completed y bank g."""
            nc.scalar.copy(y_sb[:, g * 512 : g * 512 + 256], y_ps[g][:, 0:256])
            nc.vector.tensor_copy(y_sb[:, g * 512 + 256 : g * 512 + 512], y_ps[g][:, 256:512])
            yt_ps = psum.tile([P, 4 * FO], FP, tag="yt", bufs=1, name="ps_yt")
            for q in range(4):
                nc.tensor.transpose(
                    yt_ps[:, q * FO : (q + 1) * FO],
                    y_sb[:, ts(4 * g + q, P)], ident[0:FO, 0:FO],
                )
            nc.scalar.copy(yT_sb[:, g * 4 * FO : (g + 1) * 4 * FO], yt_ps[:])
            nc.vector.reciprocal(invS3[:, 4 * g : 4 * g + 4, :], yT3[:, 4 * g : 4 * g + 4, F:FO])
            nc.vector.tensor_copy(invS_bf[:, 4 * g : 4 * g + 4], invS[:, 4 * g : 4 * g + 4])
            for t in range(4 * g, 4 * g + 4):
                nc.tensor.matmul(
                    g_ps, yT_sb[:, t * FO : t * FO + F], invS_bf[:, t : t + 1],
                    start=(t == 0), stop=(t == T - 1), skip_group_check=True,
                )

        # s1 broadcast -> g_b, all four chunks up front (gated only on hT+w1_rep)
        for g in range(4):
            s1b_ps = psum.tile([P, 512], FP, tag="s1b", bufs=1, name="ps_s1b")
            nc.tensor.matmul(s1b_ps[:], w1_rep[:], hT[:, ts(g, 512)], start=True, stop=True)
            nc.scalar.activation(g_b[:, ts(g, 512)], s1b_ps[:], AF.Exp, scale=ALPHA - 1.0)

        # [Wh_t | s2_t] slabs: one matmul per tile against [W | w2]
        for g in range(4):
            wh_ps = psum.tile([P, 4 * FO], FP, tag="wh", bufs=2, name="ps_wh")
            wh3 = wh_ps[:].rearrange("p (q c) -> p q c", c=FO)
            for q in range(4):
                t = 4 * g + q
                nc.tensor.matmul(
                    wh3[:, q, :], hT[:, ts(t, P)], Wv, start=True, stop=True
                )
            nc.scalar.copy(WhO3[:, 4 * g : 4 * g + 4, 0:F], wh3[:, :, 0:F])
            nc.scalar.activation(kcol3[:, 4 * g : 4 * g + 4, :], wh3[:, :, F:FO], AF.Exp, scale=1.0 - ALPHA)
            nc.scalar.activation(qcol3[:, 4 * g : 4 * g + 4, :], wh3[:, :, F:FO], AF.Exp, scale=ALPHA)

        # main N^2 stream: 4 chunk blocks of 16 DVE tensor_scalar + 16 PE matmuls
        for g in range(4):
            for t in range(T):
                nc.vector.tensor_scalar(
                    Ms[t][:, ts(g, 512)], g_b[:, ts(g, 512)],
                    kcol[:, t : t + 1], qcol[:, t : t + 1], OP.max, OP.mult,
                )
            for t in range(T):
                nc.tensor.matmul(
                    y_ps[g][0:FO, :], WhO3[:, t, :], Ms[t][:, ts(g, 512)],
                    start=(t == 0), stop=(t == T - 1),
                )
            if g >= 1:
                epi_bank(g - 1)

        epi_bank(3)
        pro_psum_ctx.__exit__(None, None, None)
        out_sb = small.tile([F, 1], FP, tag="out")
        nc.scalar.mul(out_sb[:], g_ps, 1.0 / N)
        nc.sync.dma_start(outd[:], out_sb[:])
        ypsum_ctx.__exit__(None, None, None)


def build(reps: int = 1):
    nc = bacc.Bacc(
        "TRN2", target_bir_lowering=False, debug=False,
        enable_asserts=False, num_devices=NCORES,
    )
    hb = nc.dram_tensor("hb", [HEXT, K], BF, kind="ExternalInput").ap()
    outd = nc.dram_tensor("out", [F, 1], FP, kind="ExternalOutput").ap()

    with tile.TileContext(nc) as tc:
        for _ in range(reps):
            emit_batch(tc, outd, hb)
    nc.compile()
    return nc


_nc_cache = {}


def _get_nc(reps: int = 1):
    if reps not in _nc_cache:
        _nc_cache[reps] = build(reps)
    return _nc_cache[reps]


def _pack_inputs(hb: np.ndarray, W: np.ndarray, a: np.ndarray) -> np.ndarray:
    """Host-side staging: h plus the packed W-side data as extra rows, bf16."""
    wp = np.zeros((K, WPC), dtype=np.float32)
    wp[:, 0:F] = W
    wp[0:F, FO : FO + K] = W.T
    wp[0:F, FO + K] = a[:F]
    wp[0:F, FO + K + 1] = a[F:]
    ext = np.zeros((HEXT, K), dtype=np.float32)
    ext[0:WPC] = wp.T
    ext[WPAD:HEXT] = hb
    return ext.astype(ml_dtypes.bfloat16)


def kernel(h: np.ndarray, W: np.ndarray, a: np.ndarray) -> np.ndarray:
    assert h.shape == (NCORES, N, K) and W.shape == (K, F) and a.shape == (2 * F,)
    nc = _get_nc(1)
    in_maps = [{"hb": _pack_inputs(h[b], W, a)} for b in range(NCORES)]
    res = run_bass_kernel_spmd(nc, in_maps, core_ids=list(range(NCORES)))
    out = np.stack([res.results[b]["out"].reshape(F) for b in range(NCORES)])
    return out.astype(np.float32)


# revision 30
# speedup vs baseline: 1.0452x; 1.0163x over previous
"""GAT layer kernel for Trainium2, data-parallel over batch across 8 NeuronCores.

Per core (one batch b), with s1 = Wh@a1, s2 = Wh@a2, alpha the LeakyReLU slope:

  E_ij = exp(leaky_relu(s1_i + s2_j)) = e^{s1_i} * max(e^{s2_j}, e^{(a-1)s1_i} e^{a s2_j})

The e^{s1_i} factor cancels in softmax(E)_ij = M_ij / S_i, where

  M'_ji = (g_i max k_j) * Q_j        g_i = e^{(a-1)s1_i}  (broadcast tensor)
                                     k_j = e^{(1-a)s2_j}, Q_j = e^{a s2_j} (scalars)
  S_i   = sum_j M'_ji

M' is laid out with partition = column node j, free = row node i, so each j-tile
chunk is ONE DVE tensor_scalar (max,mult with two per-partition scalars) in bf16
at 4x rate.  PE consumes each M' chunk with stationary [Wh_t | 1], producing the
unnormalized output y_fi = sum_j Wh_jf M'_ji and S_i in one 65-row PSUM stream.
Inputs are pre-converted to bf16 on the host; hT comes from DRAM in a single
XBAR DMA transpose.  [Wh_t | s2_t] come from one matmul per tile against the
moving operand [W | w2], w2 = W@a2.  Work runs in 4 column-chunk blocks; each y
PSUM bank completes at the end of its block, so the normalization epilogue
(out_f = (1/N) sum_i y_fi / S_i, via PE transpose + small matmuls) pipelines
per bank behind the next block.
"""
import sys
sys.path.insert(0, "/opt/trn_rl_repo")
from contextlib import ExitStack

import ml_dtypes
import numpy as np

import concourse.bass as bass
import concourse.tile as tile
from concourse import bacc, mybir
from concourse.bass_utils import run_bass_kernel_spmd
from concourse.masks import make_identity

N, K, F, P, T = 2048, 128, 64, 128, 16  # nodes, f_in, f_out, partitions, j tiles
NC_ = 4            # 512-wide column chunks per N
ALPHA = 0.2
NCORES = 8
FP = mybir.dt.float32
BF = mybir.dt.bfloat16
AF = mybir.ActivationFunctionType
OP = mybir.AluOpType
ts = bass.ts
FO = F + 1
WPC = FO + K + 2  # packed W-side input columns
WPAD = ((WPC + 15) // 16) * 16      # wp rows padded for the XBAR
HEXT = WPAD + N                     # wp rows first, then h rows
HCUT = WPAD + N // 2                # first-DMA row count (wp + h chunks 0-1)


def emit_batch(tc, outd, hb):
    """Emit the full per-batch computation (one repetition)."""
    nc = tc.nc
    with ExitStack() as ctx:
        big = ctx.enter_context(tc.tile_pool(name="big", bufs=1))
        mpool = ctx.enter_context(tc.tile_pool(name="m", bufs=1))
        small = ctx.enter_context(tc.tile_pool(name="small", bufs=1))
        ypsum_ctx = tc.tile_pool(name="yps", bufs=1, space=bass.MemorySpace.PSUM)
        ypsum = ypsum_ctx.__enter__()
        pro_psum_ctx = tc.tile_pool(name="ps", bufs=1, space=bass.MemorySpace.PSUM)
        psum = pro_psum_ctx.__enter__()

        # TWO XBAR DMA transposes deliver the packed W-side data
        # [W | w2slot | W^T | a12] (prepended as extra h rows on the host)
        # plus h chunks 0-1, then h chunks 2-3 — so the W-prep chain and the
        # first blocks start while the second half is still in flight
        hTx = big.tile([P, HEXT], BF)
        nc.sync.dma_start_transpose(hTx[:, 0:HCUT], hb[0:HCUT, :])
        nc.scalar.dma_start_transpose(hTx[:, HCUT:HEXT], hb[HCUT:HEXT, :])
        hT = hTx[:, WPAD : WPAD + N]
        Wv = hTx[:, 0:FO]                 # [W | w2] moving operand (w2 filled below)
        WT_bf = hTx[0:F, FO : FO + K]     # W^T
        a12_bf = hTx[0:F, FO + K : WPC]   # [a1 | a2]

        # junk tile: PE clock-warm operand, ACT exp-table warm, shape donor
        junk = small.tile([P, 512], BF, tag="junk")
        nc.vector.memset(junk[:], 1.0)
        warm1 = small.tile([P, 1], FP, tag="warm1")
        nc.scalar.activation(warm1[:], junk[:, 0:1], AF.Exp)
        warm_ps = psum.tile([P, 512], FP, tag="s1b", bufs=1, name="ps_warm")
        for _ in range(4):
            nc.tensor.matmul(warm_ps[:], junk[0:P, 0:P], junk[:], start=True, stop=True)

        ident = small.tile([P, P], FP, tag="ident")
        make_identity(nc, ident[:])

        # W prep: [w1|w2] = (W^T)^T @ [a1|a2], w1_rep, w2 into the Wv slot
        wprep_ps = psum.tile([P, 4 * FO], FP, tag="wh", bufs=2, name="ps_wprep")
        nc.tensor.matmul(wprep_ps[:, 0:2], WT_bf, a12_bf, start=True, stop=True)
        w12_sb = small.tile([K, 2], FP, tag="w12")
        nc.vector.tensor_copy(w12_sb[:], wprep_ps[:, 0:2])
        w1_rep = small.tile([K, P], BF, tag="w1rep")
        nc.scalar.activation(w1_rep[:], junk[0:K, 0:P], AF.Identity, bias=w12_sb[:, 0:1], scale=0.0)
        nc.vector.tensor_copy(hTx[:, F:FO], w12_sb[:, 1:2])

        g_b = big.tile([P, N], BF)      # g_b[p, i] = e^{(a-1) s1_i}  (any p)
        kcol = small.tile([P, T], FP, tag="kcol")   # k_j column layout
        qcol = small.tile([P, T], FP, tag="qcol")   # Q_j column layout
        kcol3 = kcol[:].rearrange("p (t one) -> p t one", one=1)
        qcol3 = qcol[:].rearrange("p (t one) -> p t one", one=1)
        WhO = big.tile([P, T * FO], BF)             # [Wh_t | ones] blocks
        WhO3 = WhO[:].rearrange("p (t c) -> p t c", c=FO)
        nc.vector.memset(WhO3[:, :, F:FO], 1.0)

        y_ps = [
            ypsum.tile([FO, 512], FP, tag=f"y{c}", name=f"y_ps{c}") for c in range(NC_)
        ]
        Ms = [mpool.tile([P, N], BF, tag=f"M{t}", name=f"M{t}") for t in range(T)]

        # epilogue tiles (filled per bank as each completes)
        y_sb = small.tile([FO, N], FP, tag="ysb")
        yT_sb = small.tile([P, T * FO], BF, tag="yTsb")
        yT3 = yT_sb[:].rearrange("p (t c) -> p t c", c=FO)
        invS = small.tile([P, T], FP, tag="invS")
        invS3 = invS[:].rearrange("p (t one) -> p t one", one=1)
        invS_bf = small.tile([P, T], BF, tag="invSbf")

        g_ps = y_ps[0][0:F, 511:512]

        def epi_bank(g):
            """Normalization pipeline for completed y bank g."""
            nc.scalar.copy(y_sb[:, g * 512 : g * 512 + 256], y_ps[g][:, 0:256])
            nc.vector.tensor_copy(y_sb[:, g * 512 + 256 : g * 512 + 512], y_ps[g][:, 256:512])
            yt_ps = psum.tile([P, 4 * FO], FP, tag="yt", bufs=1, name="ps_yt")
            for q in range(4):
                nc.tensor.transpose(
                    yt_ps[:, q * FO : (q + 1) * FO],
                    y_sb[:, ts(4 * g + q, P)], ident[0:FO, 0:FO],
                )
            nc.scalar.copy(yT_sb[:, g * 4 * FO : (g + 1) * 4 * FO], yt_ps[:])
            nc.vector.reciprocal(invS3[:, 4 * g : 4 * g + 4, :], yT3[:, 4 * g : 4 * g + 4, F:FO])
            nc.vector.tensor_copy(invS_bf[:, 4 * g : 4 * g + 4], invS[:, 4 * g : 4 * g + 4])
            for t in range(4 * g, 4 * g + 4):
                nc.tensor.matmul(
                    g_ps, yT_sb[:, t * FO : t * FO + F], invS_bf[:, t : t + 1],
                    start=(t == 0), stop=(t == T - 1), skip_group_check=True,
                )

        # s1 broadcast -> g_b: chunk 0 first (it alone gates block 0), rest after
        def s1b_chunk(g):
            s1b_ps = psum.tile([P, 512], FP, tag="s1b", bufs=1, name="ps_s1b")
            nc.tensor.matmul(s1b_ps[:], w1_rep[:], hT[:, ts(g, 512)], start=True, stop=True)
            nc.scalar.activation(g_b[:, ts(g, 512)], s1b_ps[:], AF.Exp, scale=ALPHA - 1.0)

        s1b_chunk(0)

        # [Wh_t | s2_t] slabs interleaved with block 0 of the N^2 stream:
        # each slab unlocks 4 tiles' chunk-0 tensor_scalar + y matmuls
        for g4 in range(4):
            wh_ps = psum.tile([P, 4 * FO], FP, tag="wh", bufs=2, name="ps_wh")
            wh3 = wh_ps[:].rearrange("p (q c) -> p q c", c=FO)
            for q in range(4):
                t = 4 * g4 + q
                nc.tensor.matmul(
                    wh3[:, q, :], hT[:, ts(t, P)], Wv, start=True, stop=True
                )
            if g4 == 0:
                nc.vector.tensor_copy(WhO3[:, 0:4, 0:F], wh3[:, :, 0:F])
            else:
                nc.scalar.copy(WhO3[:, 4 * g4 : 4 * g4 + 4, 0:F], wh3[:, :, 0:F])
            nc.scalar.activation(kcol3[:, 4 * g4 : 4 * g4 + 4, :], wh3[:, :, F:FO], AF.Exp, scale=1.0 - ALPHA)
            nc.scalar.activation(qcol3[:, 4 * g4 : 4 * g4 + 4, :], wh3[:, :, F:FO], AF.Exp, scale=ALPHA)
            if g4 == 0:
                for g in range(1, 4):
                    s1b_chunk(g)
            for t in range(4 * g4, 4 * g4 + 4):
                nc.vector.tensor_scalar(
                    Ms[t][:, 0:512], g_b[:, 0:512],
                    kcol[:, t : t + 1], qcol[:, t : t + 1], OP.max, OP.mult,
                )
            for t in range(4 * g4, 4 * g4 + 4):
                nc.tensor.matmul(
                    y_ps[0][0:FO, :], WhO3[:, t, :], Ms[t][:, 0:512],
                    start=(t == 0), stop=(t == T - 1),
                )

        # remaining chunk blocks: 16 DVE tensor_scalar + 16 PE matmuls each
        for g in range(1, 4):
            for t in range(T):
                nc.vector.tensor_scalar(
                    Ms[t][:, ts(g, 512)], g_b[:, ts(g, 512)],
                    kcol[:, t : t + 1], qcol[:, t : t + 1], OP.max, OP.mult,
                )
            for t in range(T):
                nc.tensor.matmul(
                    y_ps[g][0:FO, :], WhO3[:, t, :], Ms[t][:, ts(g, 512)],
                    start=(t == 0), stop=(t == T - 1),
                )
            epi_bank(g - 1)

        epi_bank(3)
        pro_psum_ctx.__exit__(None, None, None)
        out_sb = small.tile([F, 1], FP, tag="out")
        nc.scalar.mul(out_sb[:], g_ps, 1.0 / N)
        nc.sync.dma_start(outd[:], out_sb[:])
        ypsum_ctx.__exit__(None, None, None)


def build(reps: int = 1):
    nc = bacc.Bacc(
        "TRN2", target_bir_lowering=False, debug=False,
        enable_asserts=False, num_devices=NCORES,
    )
    hb = nc.dram_tensor("hb", [HEXT, K], BF, kind="ExternalInput").ap()
    outd = nc.dram_tensor("out", [F, 1], FP, kind="ExternalOutput").ap()

    with tile.TileContext(nc) as tc:
        for _ in range(reps):
            emit_batch(tc, outd, hb)
    nc.compile()
    return nc


_nc_cache = {}


def _get_nc(reps: int = 1):
    if reps not in _nc_cache:
        _nc_cache[reps] = build(reps)
    return _nc_cache[reps]


def _pack_inputs(hb: np.ndarray, W: np.ndarray, a: np.ndarray) -> np.ndarray:
    """Host-side staging: h plus the packed W-side data as extra rows, bf16."""
    wp = np.zeros((K, WPC), dtype=np.float32)
    wp[:, 0:F] = W
    wp[0:F, FO : FO + K] = W.T
    wp[0:F, FO + K] = a[:F]
    wp[0:F, FO + K + 1] = a[F:]
    ext = np.zeros((HEXT, K), dtype=np.float32)
    ext[0:WPC] = wp.T
    ext[WPAD:HEXT] = hb
    return ext.astype(ml_dtypes.bfloat16)


def kernel(h: np.ndarray, W: np.ndarray, a: np.ndarray) -> np.ndarray:
    assert h.shape == (NCORES, N, K) and W.shape == (K, F) and a.shape == (2 * F,)
    nc = _get_nc(1)
    in_maps = [{"hb": _pack_inputs(h[b], W, a)} for b in range(NCORES)]
    res = run_bass_kernel_spmd(nc, in_maps, core_ids=list(range(NCORES)))
    out = np.stack([res.results[b]["out"].reshape(F) for b in range(NCORES)])
    return out.astype(np.float32)


# revision 34
# speedup vs baseline: 1.0485x; 1.0032x over previous
"""GAT layer kernel for Trainium2, data-parallel over batch across 8 NeuronCores.

Per core (one batch b), with s1 = Wh@a1, s2 = Wh@a2, alpha the LeakyReLU slope:

  E_ij = exp(leaky_relu(s1_i + s2_j)) = e^{s1_i} * max(e^{s2_j}, e^{(a-1)s1_i} e^{a s2_j})

The e^{s1_i} factor cancels in softmax(E)_ij = M_ij / S_i, where

  M'_ji = (g_i max k_j) * Q_j        g_i = e^{(a-1)s1_i}  (broadcast tensor)
                                     k_j = e^{(1-a)s2_j}, Q_j = e^{a s2_j} (scalars)
  S_i   = sum_j M'_ji

M' is laid out with partition = column node j, free = row node i, so each j-tile
chunk is ONE DVE tensor_scalar (max,mult with two per-partition scalars) in bf16
at 4x rate.  PE consumes each M' chunk with stationary [Wh_t | 1], producing the
unnormalized output y_fi = sum_j Wh_jf M'_ji and S_i in one 65-row PSUM stream.
Inputs are pre-converted to bf16 on the host; hT comes from DRAM in a single
XBAR DMA transpose.  [Wh_t | s2_t] come from one matmul per tile against the
moving operand [W | w2], w2 = W@a2.  Work runs in 4 column-chunk blocks; each y
PSUM bank completes at the end of its block, so the normalization epilogue
(out_f = (1/N) sum_i y_fi / S_i, via PE transpose + small matmuls) pipelines
per bank behind the next block.
"""
import sys
sys.path.insert(0, "/opt/trn_rl_repo")
from contextlib import ExitStack

import ml_dtypes
import numpy as np

import concourse.bass as bass
import concourse.tile as tile
from concourse import bacc, mybir
from concourse.bass_utils import run_bass_kernel_spmd
from concourse.masks import make_identity

N, K, F, P, T = 2048, 128, 64, 128, 16  # nodes, f_in, f_out, partitions, j tiles
NC_ = 4            # 512-wide column chunks per N
ALPHA = 0.2
NCORES = 8
FP = mybir.dt.float32
BF = mybir.dt.bfloat16
AF = mybir.ActivationFunctionType
OP = mybir.AluOpType
ts = bass.ts
FO = F + 1
WPC = FO + K + 2  # packed W-side input columns
WPAD = ((WPC + 15) // 16) * 16      # wp rows padded for the XBAR
HEXT = WPAD + N                     # wp rows first, then h rows
HCUT = WPAD + N // 2                # first-DMA row count (wp + h chunks 0-1)


def emit_batch(tc, outd, hb):
    """Emit the full per-batch computation (one repetition)."""
    nc = tc.nc
    with ExitStack() as ctx:
        big = ctx.enter_context(tc.tile_pool(name="big", bufs=1))
        mpool = ctx.enter_context(tc.tile_pool(name="m", bufs=1))
        small = ctx.enter_context(tc.tile_pool(name="small", bufs=1))
        ypsum_ctx = tc.tile_pool(name="yps", bufs=1, space=bass.MemorySpace.PSUM)
        ypsum = ypsum_ctx.__enter__()
        pro_psum_ctx = tc.tile_pool(name="ps", bufs=1, space=bass.MemorySpace.PSUM)
        psum = pro_psum_ctx.__enter__()

        # TWO XBAR DMA transposes deliver the packed W-side data
        # [W | w2slot | W^T | a12] (prepended as extra h rows on the host)
        # plus h chunks 0-1, then h chunks 2-3 — so the W-prep chain and the
        # first blocks start while the second half is still in flight
        hTx = big.tile([P, HEXT], BF)
        nc.sync.dma_start_transpose(hTx[:, 0:HCUT], hb[0:HCUT, :])
        nc.scalar.dma_start_transpose(hTx[:, HCUT:HEXT], hb[HCUT:HEXT, :])
        hT = hTx[:, WPAD : WPAD + N]
        Wv = hTx[:, 0:FO]                 # [W | w2] moving operand (w2 filled below)
        WT_bf = hTx[0:F, FO : FO + K]     # W^T
        a12_bf = hTx[0:F, FO + K : WPC]   # [a1 | a2]

        # junk tile: PE clock-warm operand, ACT exp-table warm, shape donor
        junk = small.tile([P, 512], BF, tag="junk")
        nc.vector.memset(junk[:], 1.0)
        warm1 = small.tile([P, 1], FP, tag="warm1")
        nc.scalar.activation(warm1[:], junk[:, 0:1], AF.Exp)
        warm_ps = psum.tile([P, 512], FP, tag="s1b", bufs=1, name="ps_warm")
        for _ in range(4):
            nc.tensor.matmul(warm_ps[:], junk[0:P, 0:P], junk[:], start=True, stop=True)

        ident = small.tile([P, P], FP, tag="ident")
        make_identity(nc, ident[:])

        # W prep: [w1|w2] = (W^T)^T @ [a1|a2], w1_rep, w2 into the Wv slot
        wprep_ps = psum.tile([P, 4 * FO], FP, tag="wh", bufs=2, name="ps_wprep")
        nc.tensor.matmul(wprep_ps[:, 0:2], WT_bf, a12_bf, start=True, stop=True)
        w12_sb = small.tile([K, 2], FP, tag="w12")
        nc.vector.tensor_copy(w12_sb[:], wprep_ps[:, 0:2])
        w1_rep = small.tile([K, P], BF, tag="w1rep")
        nc.scalar.activation(w1_rep[:], junk[0:K, 0:P], AF.Identity, bias=w12_sb[:, 0:1], scale=0.0)
        nc.vector.tensor_copy(hTx[:, F:FO], w12_sb[:, 1:2])

        g_b = big.tile([P, N], BF)      # g_b[p, i] = e^{(a-1) s1_i}  (any p)
        kcol = small.tile([P, T], FP, tag="kcol")   # k_j column layout
        qcol = small.tile([P, T], FP, tag="qcol")   # Q_j column layout
        kcol3 = kcol[:].rearrange("p (t one) -> p t one", one=1)
        qcol3 = qcol[:].rearrange("p (t one) -> p t one", one=1)
        WhO = big.tile([P, T * FO], BF)             # [Wh_t | ones] blocks
        WhO3 = WhO[:].rearrange("p (t c) -> p t c", c=FO)
        nc.vector.memset(WhO3[:, :, F:FO], 1.0)

        y_ps = [
            ypsum.tile([FO, 512], FP, tag=f"y{c}", name=f"y_ps{c}") for c in range(NC_)
        ]
        Ms = [mpool.tile([P, N], BF, tag=f"M{t}", name=f"M{t}") for t in range(T)]

        # epilogue tiles (filled per bank as each completes)
        y_sb = small.tile([FO, N], FP, tag="ysb")
        yT_sb = small.tile([P, T * FO], BF, tag="yTsb")
        yT3 = yT_sb[:].rearrange("p (t c) -> p t c", c=FO)
        invS = small.tile([P, T], FP, tag="invS")
        invS3 = invS[:].rearrange("p (t one) -> p t one", one=1)
        invS_bf = small.tile([P, T], BF, tag="invSbf")

        g_ps = y_ps[0][0:F, 511:512]

        def epi_bank(g):
            """Normalization pipeline for completed y bank g."""
            nc.scalar.copy(y_sb[:, g * 512 : g * 512 + 256], y_ps[g][:, 0:256])
            nc.vector.tensor_copy(y_sb[:, g * 512 + 256 : g * 512 + 512], y_ps[g][:, 256:512])
            yt_ps = psum.tile([P, 4 * FO], FP, tag="yt", bufs=1, name="ps_yt")
            for q in range(4):
                nc.tensor.transpose(
                    yt_ps[:, q * FO : (q + 1) * FO],
                    y_sb[:, ts(4 * g + q, P)], ident[0:FO, 0:FO],
                )
            nc.scalar.copy(yT_sb[:, g * 4 * FO : (g + 1) * 4 * FO], yt_ps[:])
            nc.vector.reciprocal(invS3[:, 4 * g : 4 * g + 4, :], yT3[:, 4 * g : 4 * g + 4, F:FO])
            nc.vector.tensor_copy(invS_bf[:, 4 * g : 4 * g + 4], invS[:, 4 * g : 4 * g + 4])
            for t in range(4 * g, 4 * g + 4):
                nc.tensor.matmul(
                    g_ps, yT_sb[:, t * FO : t * FO + F], invS_bf[:, t : t + 1],
                    start=(t == 0), stop=(t == T - 1), skip_group_check=True,
                )

        # s1 broadcast -> g_b: chunk 0 first (it alone gates block 0), rest after
        def s1b_chunk(g):
            s1b_ps = psum.tile([P, 512], FP, tag="s1b", bufs=1, name="ps_s1b")
            nc.tensor.matmul(s1b_ps[:], w1_rep[:], hT[:, ts(g, 512)], start=True, stop=True)
            nc.scalar.activation(g_b[:, ts(g, 512)], s1b_ps[:], AF.Exp, scale=ALPHA - 1.0)

        s1b_chunk(0)

        # [Wh_t | s2_t] slabs interleaved with block 0 of the N^2 stream:
        # each slab unlocks 4 tiles' chunk-0 tensor_scalar + y matmuls
        for g4 in range(4):
            wh_ps = psum.tile([P, 4 * FO], FP, tag="wh", bufs=2, name="ps_wh")
            wh3 = wh_ps[:].rearrange("p (q c) -> p q c", c=FO)
            for q in range(4):
                t = 4 * g4 + q
                nc.tensor.matmul(
                    wh3[:, q, :], hT[:, ts(t, P)], Wv, start=True, stop=True
                )
            nc.scalar.activation(kcol3[:, 4 * g4 : 4 * g4 + 4, :], wh3[:, :, F:FO], AF.Exp, scale=1.0 - ALPHA)
            nc.scalar.activation(qcol3[:, 4 * g4 : 4 * g4 + 4, :], wh3[:, :, F:FO], AF.Exp, scale=ALPHA)
            if g4 == 0:
                nc.vector.tensor_copy(WhO3[:, 0:4, 0:F], wh3[:, :, 0:F])
            else:
                nc.scalar.copy(WhO3[:, 4 * g4 : 4 * g4 + 4, 0:F], wh3[:, :, 0:F])
            if g4 == 0:
                for g in range(1, 4):
                    s1b_chunk(g)
            for t in range(4 * g4, 4 * g4 + 4):
                nc.vector.tensor_scalar(
                    Ms[t][:, 0:512], g_b[:, 0:512],
                    kcol[:, t : t + 1], qcol[:, t : t + 1], OP.max, OP.mult,
                )
            for t in range(4 * g4, 4 * g4 + 4):
                nc.tensor.matmul(
                    y_ps[0][0:FO, :], WhO3[:, t, :], Ms[t][:, 0:512],
                    start=(t == 0), stop=(t == T - 1),
                )

        # remaining chunk blocks: 16 DVE tensor_scalar + 16 PE matmuls each
        for g in range(1, 4):
            for t in range(T):
                nc.vector.tensor_scalar(
                    Ms[t][:, ts(g, 512)], g_b[:, ts(g, 512)],
                    kcol[:, t : t + 1], qcol[:, t : t + 1], OP.max, OP.mult,
                )
            for t in range(T):
                nc.tensor.matmul(
                    y_ps[g][0:FO, :], WhO3[:, t, :], Ms[t][:, ts(g, 512)],
                    start=(t == 0), stop=(t == T - 1),
                )
            epi_bank(g - 1)

        epi_bank(3)
        pro_psum_ctx.__exit__(None, None, None)
        out_sb = small.tile([F, 1], FP, tag="out")
        nc.scalar.mul(out_sb[:], g_ps, 1.0 / N)
        nc.sync.dma_start(outd[:], out_sb[:])
        ypsum_ctx.__exit__(None, None, None)


def build(reps: int = 1):
    nc = bacc.Bacc(
        "TRN2", target_bir_lowering=False, debug=False,
        enable_asserts=False, num_devices=NCORES,
    )
    hb = nc.dram_tensor("hb", [HEXT, K], BF, kind="ExternalInput").ap()
    outd = nc.dram_tensor("out", [F, 1], FP, kind="ExternalOutput").ap()

    with tile.TileContext(nc) as tc:
        for _ in range(reps):
            emit_batch(tc, outd, hb)
    nc.compile()
    return nc


_nc_cache = {}


def _get_nc(reps: int = 1):
    if reps not in _nc_cache:
        _nc_cache[reps] = build(reps)
    return _nc_cache[reps]


def _pack_inputs(hb: np.ndarray, W: np.ndarray, a: np.ndarray) -> np.ndarray:
    """Host-side staging: h plus the packed W-side data as extra rows, bf16."""
    wp = np.zeros((K, WPC), dtype=np.float32)
    wp[:, 0:F] = W
    wp[0:F, FO : FO + K] = W.T
    wp[0:F, FO + K] = a[:F]
    wp[0:F, FO + K + 1] = a[F:]
    ext = np.zeros((HEXT, K), dtype=np.float32)
    ext[0:WPC] = wp.T
    ext[WPAD:HEXT] = hb
    return ext.astype(ml_dtypes.bfloat16)


def kernel(h: np.ndarray, W: np.ndarray, a: np.ndarray) -> np.ndarray:
    assert h.shape == (NCORES, N, K) and W.shape == (K, F) and a.shape == (2 * F,)
    nc = _get_nc(1)
    in_maps = [{"hb": _pack_inputs(h[b], W, a)} for b in range(NCORES)]
    res = run_bass_kernel_spmd(nc, in_maps, core_ids=list(range(NCORES)))
    out = np.stack([res.results[b]["out"].reshape(F) for b in range(NCORES)])
    return out.astype(np.float32)


# revision 42
# speedup vs baseline: 1.0530x; 1.0043x over previous
"""GAT layer kernel for Trainium2, data-parallel over batch across 8 NeuronCores.

Per core (one batch b), with s1 = Wh@a1, s2 = Wh@a2, alpha the LeakyReLU slope:

  E_ij = exp(leaky_relu(s1_i + s2_j)) = e^{s1_i} * max(e^{s2_j}, e^{(a-1)s1_i} e^{a s2_j})

The e^{s1_i} factor cancels in softmax(E)_ij = M_ij / S_i, where

  M'_ji = (g_i max k_j) * Q_j        g_i = e^{(a-1)s1_i}  (broadcast tensor)
                                     k_j = e^{(1-a)s2_j}, Q_j = e^{a s2_j} (scalars)
  S_i   = sum_j M'_ji

M' is laid out with partition = column node j, free = row node i, so each j-tile
chunk is ONE DVE tensor_scalar (max,mult with two per-partition scalars) in bf16
at 4x rate.  PE consumes each M' chunk with stationary [Wh_t | 1], producing the
unnormalized output y_fi = sum_j Wh_jf M'_ji and S_i in one 65-row PSUM stream.
Inputs are pre-converted to bf16 on the host; hT comes from DRAM in a single
XBAR DMA transpose.  [Wh_t | s2_t] come from one matmul per tile against the
moving operand [W | w2], w2 = W@a2.  Work runs in 4 column-chunk blocks; each y
PSUM bank completes at the end of its block, so the normalization epilogue
(out_f = (1/N) sum_i y_fi / S_i, via PE transpose + small matmuls) pipelines
per bank behind the next block.
"""
import sys
sys.path.insert(0, "/opt/trn_rl_repo")
from contextlib import ExitStack

import ml_dtypes
import numpy as np

import concourse.bass as bass
import concourse.tile as tile
from concourse import bacc, mybir
from concourse.bass_utils import run_bass_kernel_spmd
from concourse.masks import make_identity

N, K, F, P, T = 2048, 128, 64, 128, 16  # nodes, f_in, f_out, partitions, j tiles
NC_ = 4            # 512-wide column chunks per N
ALPHA = 0.2
NCORES = 8
FP = mybir.dt.float32
BF = mybir.dt.bfloat16
AF = mybir.ActivationFunctionType
OP = mybir.AluOpType
ts = bass.ts
FO = F + 1
WPC = FO + K + 2  # packed W-side input columns
WPAD = ((WPC + 15) // 16) * 16      # wp rows padded for the XBAR
HEXT = WPAD + N                     # wp rows first, then h rows
HCUT = WPAD + N // 4                # first-DMA row count (wp + h chunk 0)


def emit_batch(tc, outd, hb):
    """Emit the full per-batch computation (one repetition)."""
    nc = tc.nc
    with ExitStack() as ctx:
        big = ctx.enter_context(tc.tile_pool(name="big", bufs=1))
        mpool = ctx.enter_context(tc.tile_pool(name="m", bufs=1))
        small = ctx.enter_context(tc.tile_pool(name="small", bufs=1))
        ypsum_ctx = tc.tile_pool(name="yps", bufs=1, space=bass.MemorySpace.PSUM)
        ypsum = ypsum_ctx.__enter__()
        pro_psum_ctx = tc.tile_pool(name="ps", bufs=1, space=bass.MemorySpace.PSUM)
        psum = pro_psum_ctx.__enter__()

        # TWO XBAR DMA transposes deliver the packed W-side data
        # [W | w2slot | W^T | a12] (prepended as extra h rows on the host)
        # plus h chunks 0-1, then h chunks 2-3 — so the W-prep chain and the
        # first blocks start while the second half is still in flight
        hTx = big.tile([P, HEXT], BF)
        nc.sync.dma_start_transpose(hTx[:, 0:HCUT], hb[0:HCUT, :])
        nc.scalar.dma_start_transpose(hTx[:, HCUT:HEXT], hb[HCUT:HEXT, :])
        hT = hTx[:, WPAD : WPAD + N]
        Wv = hTx[:, 0:FO]                 # [W | w2] moving operand (w2 filled below)
        WT_bf = hTx[0:F, FO : FO + K]     # W^T
        a12_bf = hTx[0:F, FO + K : WPC]   # [a1 | a2]

        # junk tile: PE clock-warm operand, ACT exp-table warm, shape donor
        junk = small.tile([P, 512], BF, tag="junk")
        nc.vector.memset(junk[:], 1.0)
        warm1 = small.tile([P, 1], FP, tag="warm1")
        nc.scalar.activation(warm1[:], junk[:, 0:1], AF.Exp)
        warm_ps = psum.tile([P, 512], FP, tag="s1b", bufs=1, name="ps_warm")
        for _ in range(4):
            nc.tensor.matmul(warm_ps[:], junk[0:P, 0:P], junk[:], start=True, stop=True)

        ident = small.tile([P, P], FP, tag="ident")
        make_identity(nc, ident[:])

        # W prep: [w1|w2] = (W^T)^T @ [a1|a2], w1_rep, w2 into the Wv slot
        wprep_ps = psum.tile([P, 4 * FO], FP, tag="wh", bufs=2, name="ps_wprep")
        nc.tensor.matmul(wprep_ps[:, 0:2], WT_bf, a12_bf, start=True, stop=True)
        w12_sb = small.tile([K, 2], FP, tag="w12")
        nc.vector.tensor_copy(w12_sb[:], wprep_ps[:, 0:2])
        w1_rep = small.tile([K, P], BF, tag="w1rep")
        nc.scalar.activation(w1_rep[:], junk[0:K, 0:P], AF.Identity, bias=w12_sb[:, 0:1], scale=0.0)
        nc.vector.tensor_copy(hTx[:, F:FO], w12_sb[:, 1:2])

        g_b = big.tile([P, N], BF)      # g_b[p, i] = e^{(a-1) s1_i}  (any p)
        kcol = small.tile([P, T], FP, tag="kcol")   # k_j column layout
        qcol = small.tile([P, T], FP, tag="qcol")   # Q_j column layout
        kcol3 = kcol[:].rearrange("p (t one) -> p t one", one=1)
        qcol3 = qcol[:].rearrange("p (t one) -> p t one", one=1)
        WhO = big.tile([P, T * FO], BF)             # [Wh_t | ones] blocks
        WhO3 = WhO[:].rearrange("p (t c) -> p t c", c=FO)
        nc.vector.memset(WhO3[:, :, F:FO], 1.0)

        y_ps = [
            ypsum.tile([FO, 512], FP, tag=f"y{c}", name=f"y_ps{c}") for c in range(NC_)
        ]
        Ms = [mpool.tile([P, N], BF, tag=f"M{t}", name=f"M{t}") for t in range(T)]

        # epilogue tiles (filled per bank as each completes)
        y_sb = small.tile([FO, N], FP, tag="ysb")
        yT_sb = small.tile([P, T * FO], BF, tag="yTsb")
        yT3 = yT_sb[:].rearrange("p (t c) -> p t c", c=FO)
        invS = small.tile([P, T], FP, tag="invS")
        invS3 = invS[:].rearrange("p (t one) -> p t one", one=1)
        invS_bf = small.tile([P, T], BF, tag="invSbf")

        g_ps = y_ps[0][0:F, 511:512]

        def epi_bank(g):
            """Normalization pipeline for completed y bank g."""
            nc.scalar.copy(y_sb[:, g * 512 : g * 512 + 256], y_ps[g][:, 0:256])
            nc.vector.tensor_copy(y_sb[:, g * 512 + 256 : g * 512 + 512], y_ps[g][:, 256:512])
            yt_ps = psum.tile([P, 4 * FO], FP, tag="yt", bufs=1, name="ps_yt")
            for q in range(4):
                nc.tensor.transpose(
                    yt_ps[:, q * FO : (q + 1) * FO],
                    y_sb[:, ts(4 * g + q, P)], ident[0:FO, 0:FO],
                )
            nc.scalar.copy(yT_sb[:, g * 4 * FO : (g + 1) * 4 * FO], yt_ps[:])
            nc.vector.reciprocal(invS3[:, 4 * g : 4 * g + 4, :], yT3[:, 4 * g : 4 * g + 4, F:FO])
            nc.vector.tensor_copy(invS_bf[:, 4 * g : 4 * g + 4], invS[:, 4 * g : 4 * g + 4])
            for t in range(4 * g, 4 * g + 4):
                nc.tensor.matmul(
                    g_ps, yT_sb[:, t * FO : t * FO + F], invS_bf[:, t : t + 1],
                    start=(t == 0), stop=(t == T - 1), skip_group_check=True,
                )

        # s1 broadcast -> g_b: chunk 0 first (it alone gates block 0), rest after
        def s1b_chunk(g):
            s1b_ps = psum.tile([P, 512], FP, tag="s1b", bufs=1, name="ps_s1b")
            nc.tensor.matmul(s1b_ps[:], w1_rep[:], hT[:, ts(g, 512)], start=True, stop=True)
            nc.scalar.activation(g_b[:, ts(g, 512)], s1b_ps[:], AF.Exp, scale=ALPHA - 1.0)

        s1b_chunk(0)

        # [Wh_t | s2_t] slabs interleaved with block 0 of the N^2 stream:
        # each slab unlocks 4 tiles' chunk-0 tensor_scalar + y matmuls
        for g4 in range(4):
            wh_ps = psum.tile([P, 4 * FO], FP, tag="wh", bufs=2, name="ps_wh")
            wh3 = wh_ps[:].rearrange("p (q c) -> p q c", c=FO)
            for q in range(4):
                t = 4 * g4 + q
                nc.tensor.matmul(
                    wh3[:, q, :], hT[:, ts(t, P)], Wv, start=True, stop=True
                )
            nc.scalar.activation(kcol3[:, 4 * g4 : 4 * g4 + 4, :], wh3[:, :, F:FO], AF.Exp, scale=1.0 - ALPHA)
            nc.scalar.activation(qcol3[:, 4 * g4 : 4 * g4 + 4, :], wh3[:, :, F:FO], AF.Exp, scale=ALPHA)
            if g4 == 0:
                nc.vector.tensor_copy(WhO3[:, 0:4, 0:F], wh3[:, :, 0:F])
            else:
                nc.scalar.copy(WhO3[:, 4 * g4 : 4 * g4 + 4, 0:F], wh3[:, :, 0:F])
            if g4 == 0:
                for g in range(1, 4):
                    s1b_chunk(g)
            for t in range(4 * g4, 4 * g4 + 4):
                nc.vector.tensor_scalar(
                    Ms[t][:, 0:512], g_b[:, 0:512],
                    kcol[:, t : t + 1], qcol[:, t : t + 1], OP.max, OP.mult,
                )
            for t in range(4 * g4, 4 * g4 + 4):
                nc.tensor.matmul(
                    y_ps[0][0:FO, :], WhO3[:, t, :], Ms[t][:, 0:512],
                    start=(t == 0), stop=(t == T - 1),
                )

        # remaining chunk blocks: 16 DVE tensor_scalar + 16 PE matmuls each
        for g in range(1, 4):
            for t in range(T):
                nc.vector.tensor_scalar(
                    Ms[t][:, ts(g, 512)], g_b[:, ts(g, 512)],
                    kcol[:, t : t + 1], qcol[:, t : t + 1], OP.max, OP.mult,
                )
            for t in range(T):
                nc.tensor.matmul(
                    y_ps[g][0:FO, :], WhO3[:, t, :], Ms[t][:, ts(g, 512)],
                    start=(t == 0), stop=(t == T - 1),
                )
            epi_bank(g - 1)

        epi_bank(3)
        pro_psum_ctx.__exit__(None, None, None)
        out_sb = small.tile([F, 1], FP, tag="out")
        nc.scalar.mul(out_sb[:], g_ps, 1.0 / N)
        nc.sync.dma_start(outd[:], out_sb[:])
        ypsum_ctx.__exit__(None, None, None)


def build(reps: int = 1):
    nc = bacc.Bacc(
        "TRN2", target_bir_lowering=False, debug=False,
        enable_asserts=False, num_devices=NCORES,
    )
    hb = nc.dram_tensor("hb", [HEXT, K], BF, kind="ExternalInput").ap()
    outd = nc.dram_tensor("out", [F, 1], FP, kind="ExternalOutput").ap()

    with tile.TileContext(nc) as tc:
        for _ in range(reps):
            emit_batch(tc, outd, hb)
    nc.compile()
    return nc


_nc_cache = {}


def _get_nc(reps: int = 1):
    if reps not in _nc_cache:
        _nc_cache[reps] = build(reps)
    return _nc_cache[reps]


def _pack_inputs(hb: np.ndarray, W: np.ndarray, a: np.ndarray) -> np.ndarray:
    """Host-side staging: h plus the packed W-side data as extra rows, bf16."""
    wp = np.zeros((K, WPC), dtype=np.float32)
    wp[:, 0:F] = W
    wp[0:F, FO : FO + K] = W.T
    wp[0:F, FO + K] = a[:F]
    wp[0:F, FO + K + 1] = a[F:]
    ext = np.zeros((HEXT, K), dtype=np.float32)
    ext[0:WPC] = wp.T
    ext[WPAD:HEXT] = hb
    return ext.astype(ml_dtypes.bfloat16)


def kernel(h: np.ndarray, W: np.ndarray, a: np.ndarray) -> np.ndarray:
    assert h.shape == (NCORES, N, K) and W.shape == (K, F) and a.shape == (2 * F,)
    nc = _get_nc(1)
    in_maps = [{"hb": _pack_inputs(h[b], W, a)} for b in range(NCORES)]
    res = run_bass_kernel_spmd(nc, in_maps, core_ids=list(range(NCORES)))
    out = np.stack([res.results[b]["out"].reshape(F) for b in range(NCORES)])
    return out.astype(np.float32)
